# revision 1
# baseline (speedup 1.0000x reference)
"""BayesianLinear forward kernel for 8x Trainium2 NeuronCores.

out[b,o] = sum_i (mu[o,i] + std[o,i]*eps_w[b,o,i]) * x[b,i]
           + bias_mu[o] + bias_std[o]*eps_b[b,o]

Shapes (full): x (1024,512) f32, eps_w (1024,512,512) f32, eps_b (1024,512) f32,
weight_mu/logvar (512,512) f32, bias_mu/logvar (512,) f32 -> out (1024,512) f32.

Strategy: data-parallel over batch (128 rows/core). The 1 GiB eps_w stream is
the cost driver (memory-bound). Per core and per batch row b:
  1. DMA eps_w[b] (512x512) into SBUF in natural layout (o on partitions).
  2. TensorE transposes the 16 128x128 blocks into PSUM (i on partitions).
  3. VectorE multiplies the transposed blocks by stdT (= exp(.5*logvar)
     transposed, precomputed once) straight out of PSUM into SBUF t'.
  4. TensorE matvec: out2[b,:] accumulates lhsT = x-column (128x1 stationary),
     rhs = t' chunks -> PSUM row (1,512), fp32r for single-pass speed.
  5. ScalarE copies the PSUM row into a partition-0 staging buffer; every 16
     rows one SBUF->SBUF DMA scatters the staging buffer to 16 partitions of
     the gather tile G.
Once per core: out1 = x @ mu^T + bias_mu via 5 batched matmuls into a (b,o)
PSUM tile; bias_std broadcast via a K=1 ones matmul; U = out1 + bias_mu
+ bias_std*eps_b on VectorE.  Final: out = G + U, one DMA to HBM.
"""

import os
import sys

import numpy as np

for _p in ("/opt/trn_rl_repo", "/root/.axon_site/_ro/trn_rl_repo"):
    if os.path.isdir(_p) and _p not in sys.path:
        sys.path.insert(0, _p)

from concourse import bacc, bass, mybir  # noqa: E402
from concourse import tile  # noqa: E402
from concourse.bass_utils import run_bass_kernel_spmd  # noqa: E402
from concourse.masks import make_identity  # noqa: E402

P = 128          # partitions
I = 512          # in_features
O = 512          # out_features
B_FULL = 1024    # full batch
N_CORES = 8
B = B_FULL // N_CORES   # batch rows per core
KC = I // P      # i-chunks
OC = O // P      # o-chunks
F32 = mybir.dt.float32
F32R = mybir.dt.float32r

# dtype used for the per-b matvec (accuracy/speed knob):
#   F32R = single-pass (1 cyc/row), F32 = exact 2-pass (4 cyc/row)
MATVEC_DT = F32R
# dtype tag for the eps block transposes (pure data movement):
TRANSPOSE_DT = F32R  # 1.5 cyc/row vs 2.0 for plain fp32

STAGE_CHUNK = 16  # b-rows per staging buffer / scatter DMA


def _build_program():
    nc = bacc.Bacc("TRN2", target_bir_lowering=False, debug=False)

    x_s = nc.dram_tensor("x_s", [B, I], F32, kind="ExternalInput")
    eps_w_s = nc.dram_tensor("eps_w_s", [B, O, I], TRANSPOSE_DT,
                             kind="ExternalInput")
    eps_b_s = nc.dram_tensor("eps_b_s", [B, O], F32, kind="ExternalInput")
    w_mu = nc.dram_tensor("w_mu", [O, I], F32, kind="ExternalInput")
    w_lv = nc.dram_tensor("w_lv", [O, I], F32, kind="ExternalInput")
    b_mu = nc.dram_tensor("b_mu", [1, O], F32, kind="ExternalInput")
    b_lv = nc.dram_tensor("b_lv", [1, O], F32, kind="ExternalInput")
    out_s = nc.dram_tensor("out_s", [B, O], F32, kind="ExternalOutput")

    # Static PSUM: 3 double-bank transpose-staging tensors cycled over
    # half-iterations (n % 3) plus 2 single-bank matvec-row tensors (b % 2).
    # Static allocation avoids Tile pool slot-release waits, which would
    # otherwise push PE matmuls over the 1-sync-wait ISA limit.
    pst_st = [nc.alloc_psum_tensor(f"pst{j}", [P, 2 * O], TRANSPOSE_DT)
              for j in range(3)]
    prow_st = [nc.alloc_psum_tensor(f"prow{j}", [P, O], F32) for j in range(2)]

    with tile.TileContext(nc) as tc:
        with (
            tc.tile_pool(name="consts", bufs=1) as consts,
            tc.tile_pool(name="eps_pool", bufs=4) as eps_pool,
            tc.tile_pool(name="tp_pool", bufs=2) as tp_pool,
            tc.tile_pool(name="stage_pool", bufs=2) as stage_pool,
        ):
            ident = consts.tile([P, P], F32)
            make_identity(nc, ident[:])
            ident_r = consts.tile([P, P], TRANSPOSE_DT)
            nc.vector.tensor_copy(out=ident_r[:], in_=ident[:])

            # ---- constants / preamble ----------------------------------
            lv_sb = consts.tile([P, OC * I], F32)    # raw logvar [o_loc, c*I+i]
            std_sb = consts.tile([P, OC * I], F32)   # exp(0.5*lv)
            mu_sb = consts.tile([P, OC * I], F32)
            stdT = consts.tile([P, KC * O], F32)     # [i_loc, k*O + o]
            muT = consts.tile([P, KC * O], F32)
            x_sb = consts.tile([P, I], F32)          # [b, i]
            xT = consts.tile([P, KC * P], F32)       # [i_loc, k*P + b]
            xTr = consts.tile([P, KC * P], MATVEC_DT)  # rounded copy for matvec
            epsb_sb = consts.tile([P, O], F32)       # [b, o]
            bmu_row = consts.tile([1, O], F32)
            blv_row = consts.tile([1, O], F32)
            bstd_row = consts.tile([1, O], F32)
            ones_col = consts.tile([1, P], F32)
            U = consts.tile([P, O], F32)             # out1 + bias terms, [b, o]
            G = consts.tile([P, O], F32)             # gathered eps-term rows
            final_sb = consts.tile([P, O], F32)

            nc.sync.dma_start(out=x_sb[:], in_=x_s.ap())
            nc.sync.dma_start(out=epsb_sb[:], in_=eps_b_s.ap())
            nc.sync.dma_start(out=bmu_row[:], in_=b_mu.ap())
            nc.sync.dma_start(out=blv_row[:], in_=b_lv.ap())
            nc.sync.dma_start(
                out=lv_sb[:].rearrange("p (c i) -> p c i", c=OC),
                in_=w_lv.ap().rearrange("(c p) i -> p c i", p=P),
            )
            nc.sync.dma_start(
                out=mu_sb[:].rearrange("p (c i) -> p c i", c=OC),
                in_=w_mu.ap().rearrange("(c p) i -> p c i", p=P),
            )
            nc.vector.memset(ones_col[:], 1.0)

            # std = exp(0.5 * logvar), bias_std likewise (not in place, so
            # consumers depend on one producer engine only)
            nc.scalar.activation(std_sb[:], lv_sb[:],
                                 mybir.ActivationFunctionType.Exp, scale=0.5)
            nc.scalar.activation(bstd_row[:], blv_row[:],
                                 mybir.ActivationFunctionType.Exp, scale=0.5)

            # transpose std and mu: [o, i] -> [i, o] chunks
            ps_stage = pst_st[0].ap().bitcast(F32)
            for src, dst in ((std_sb, stdT), (mu_sb, muT)):
                for k in range(KC):
                    ps = ps_stage[:, (k % 2) * O:(k % 2) * O + O]
                    for c in range(OC):
                        nc.tensor.transpose(
                            out=ps[:, c * P:(c + 1) * P],
                            in_=src[:, c * I + k * P: c * I + (k + 1) * P],
                            identity=ident[:],
                        )
                    nc.vector.tensor_copy(out=dst[:, k * O:(k + 1) * O], in_=ps)

            # transpose x: [b, i] -> [i_loc, k*P + b]
            psx = pst_st[2].ap().bitcast(F32)[:, :KC * P]
            for k in range(KC):
                nc.tensor.transpose(
                    out=psx[:, k * P:(k + 1) * P],
                    in_=x_sb[:, k * P:(k + 1) * P],
                    identity=ident[:],
                )
            nc.vector.tensor_copy(out=xT[:], in_=psx)
            nc.vector.tensor_copy(out=xTr[:], in_=xT[:])

            # out1[b,o] = sum_i x[b,i]*mu[o,i]  (+ bias_mu via K=1 matmul)
            ps_u = pst_st[1].ap().bitcast(F32)[:, :O]
            for k in range(KC):
                nc.tensor.matmul(
                    out=ps_u,
                    lhsT=xT[:, k * P:(k + 1) * P],
                    rhs=muT[:, k * O:(k + 1) * O],
                    start=(k == 0), stop=False,
                )
            nc.tensor.matmul(out=ps_u, lhsT=ones_col[:], rhs=bmu_row[:],
                             start=False, stop=True)

            # broadcast bias_std across partitions, then
            # U = out1 + bias_mu + bias_std * eps_b
            ps_b = pst_st[1].ap().bitcast(F32)[:, O:2 * O]
            nc.tensor.matmul(out=ps_b, lhsT=ones_col[:], rhs=bstd_row[:],
                             start=True, stop=True)
            nc.vector.tensor_tensor(out=U[:], in0=epsb_sb[:], in1=ps_b,
                                    op=mybir.AluOpType.mult)
            nc.vector.tensor_tensor(out=U[:], in0=U[:], in1=ps_u,
                                    op=mybir.AluOpType.add)

            # ---- main loop over batch rows -----------------------------
            def emit_main(_iv=None):
              stage = None
              n_loop = int(os.environ.get("KERNEL_NB", B))
              for b in range(n_loop):
                eps_t = eps_pool.tile([P, OC * I], TRANSPOSE_DT, tag="eps")
                eng = nc.sync if b % 2 == 0 else nc.scalar
                eng.dma_start(
                    out=eps_t[:].rearrange("p (c i) -> p c i", c=OC),
                    in_=eps_w_s.ap()[b].rearrange("(c p) i -> p c i", p=P),
                )

                t_p = tp_pool.tile([P, KC * O], MATVEC_DT, tag="tp")
                for h in range(2):  # two halves of the i-chunks
                    n = 2 * b + h
                    pst = pst_st[n % 3].ap()
                    for kk in range(2):
                        k = 2 * h + kk
                        for c in range(OC):
                            nc.tensor.transpose(
                                out=pst[:, kk * O + c * P: kk * O + (c + 1) * P],
                                in_=eps_t[:, c * I + k * P: c * I + (k + 1) * P],
                                identity=ident_r[:],
                            )
                    nc.vector.tensor_tensor(
                        out=t_p[:, h * 2 * O:(h + 1) * 2 * O],
                        in0=pst,
                        in1=stdT[:, h * 2 * O:(h + 1) * 2 * O],
                        op=mybir.AluOpType.mult,
                    )

                prow = prow_st[b % 2].ap()[:1, :]
                for k in range(KC):
                    nc.tensor.matmul(
                        out=prow,
                        lhsT=xTr[:, k * P + b: k * P + b + 1],
                        rhs=t_p[:, k * O:(k + 1) * O],
                        start=(k == 0), stop=(k == KC - 1),
                    )

                ci = b % STAGE_CHUNK
                if ci == 0:
                    stage = stage_pool.tile([1, STAGE_CHUNK * O], F32, tag="stage")
                nc.scalar.copy(
                    out=stage[0:1, ci * O:(ci + 1) * O], in_=prow)
                if ci == STAGE_CHUNK - 1:
                    g0 = b - (STAGE_CHUNK - 1)
                    nc.scalar.dma_start(
                        out=G[g0:g0 + STAGE_CHUNK, :],
                        in_=stage[0:1, :],
                    )
                if b % 32 == 31:
                    f0 = b - 31
                    nc.vector.tensor_tensor(
                        out=final_sb[f0:f0 + 32, :],
                        in0=G[f0:f0 + 32, :],
                        in1=U[f0:f0 + 32, :],
                        op=mybir.AluOpType.add,
                    )
                    nc.scalar.dma_start(
                        out=out_s.ap()[f0:f0 + 32, :],
                        in_=final_sb[f0:f0 + 32, :],
                    )


            repeat = int(os.environ.get("KERNEL_REPEAT", "0"))
            if repeat > 1:
                with tc.For_i(0, repeat, 1):
                    emit_main()
            else:
                emit_main()

    nc.compile()
    return nc


_NC = None


def _get_program():
    global _NC
    if _NC is None:
        _NC = _build_program()
    return _NC


def kernel(**inputs) -> np.ndarray:
    x = np.ascontiguousarray(np.asarray(inputs["x"], dtype=np.float32))
    eps_w = np.ascontiguousarray(np.asarray(inputs["eps_w"], dtype=np.float32))
    eps_b = np.ascontiguousarray(np.asarray(inputs["eps_b"], dtype=np.float32))
    w_mu = np.ascontiguousarray(np.asarray(inputs["weight_mu"], dtype=np.float32))
    w_lv = np.ascontiguousarray(np.asarray(inputs["weight_logvar"], dtype=np.float32))
    b_mu = np.ascontiguousarray(
        np.asarray(inputs["bias_mu"], dtype=np.float32).reshape(1, O))
    b_lv = np.ascontiguousarray(
        np.asarray(inputs["bias_logvar"], dtype=np.float32).reshape(1, O))

    nc = _get_program()
    in_maps = []
    for ci in range(N_CORES):
        sl = slice(ci * B, (ci + 1) * B)
        in_maps.append({
            "x_s": x[sl],
            "eps_w_s": eps_w[sl],
            "eps_b_s": eps_b[sl],
            "w_mu": w_mu,
            "w_lv": w_lv,
            "b_mu": b_mu,
            "b_lv": b_lv,
        })

    res = run_bass_kernel_spmd(nc, in_maps, core_ids=list(range(N_CORES)))
    out = np.concatenate([res.results[ci]["out_s"] for ci in range(N_CORES)],
                         axis=0)
    return out.astype(np.float32)



# revision 2
# speedup vs baseline: 85.4557x; 85.4557x over previous
"""BayesianLinear forward kernel for 8x Trainium2 NeuronCores.

out[b,o] = sum_i (mu[o,i] + std[o,i]*eps_w[b,o,i]) * x[b,i]
           + bias_mu[o] + bias_std[o]*eps_b[b,o]

Shapes (full): x (1024,512) f32, eps_w (1024,512,512) f32, eps_b (1024,512) f32,
weight_mu/logvar (512,512) f32, bias_mu/logvar (512,) f32 -> out (1024,512) f32.

Strategy: data-parallel over batch (128 rows/core). The 1 GiB eps_w stream is
the cost driver (memory-bound). Per core and per batch row b:
  1. DMA eps_w[b] (512x512) into SBUF in natural layout (o on partitions),
     batched BB rows per transfer, alternating the two HWDGE rings.
  2. TensorE transposes the 16 128x128 blocks into PSUM (i on partitions).
  3. VectorE multiplies the transposed blocks by stdT (= exp(.5*logvar)
     transposed, precomputed once) straight out of PSUM into SBUF t'.
  4. TensorE matvec: out2[b,:] accumulates lhsT = x-column (128x1 stationary),
     rhs = t' chunks -> PSUM row (1,512), fp32r for single-pass speed.
  5. ScalarE copies the PSUM row into a partition-0 staging buffer; every 16
     rows one SBUF->SBUF DMA (SWDGE/gpsimd ring) scatters the staging buffer
     to 16 partitions of the gather tile G.
Once per core: out1 = x @ mu^T + bias_mu via 5 batched matmuls into a (b,o)
PSUM tile; bias_std broadcast via a K=1 ones matmul; U = out1 + bias_mu
+ bias_std*eps_b on VectorE.  Final: out = G + U, one DMA to HBM.
"""

import os
import sys

import numpy as np

for _p in ("/opt/trn_rl_repo", "/root/.axon_site/_ro/trn_rl_repo"):
    if os.path.isdir(_p) and _p not in sys.path:
        sys.path.insert(0, _p)

from concourse import bacc, bass, mybir  # noqa: E402
from concourse import tile  # noqa: E402
from concourse.bass_utils import run_bass_kernel_spmd  # noqa: E402
from concourse.masks import make_identity  # noqa: E402

P = 128          # partitions
I = 512          # in_features
O = 512          # out_features
B_FULL = 1024    # full batch
N_CORES = 8
B = B_FULL // N_CORES   # batch rows per core
KC = I // P      # i-chunks
OC = O // P      # o-chunks
F32 = mybir.dt.float32
F32R = mybir.dt.float32r

# dtype used for the per-b matvec (accuracy/speed knob):
#   F32R = single-pass (1 cyc/row), F32 = exact 2-pass (4 cyc/row)
MATVEC_DT = F32R
# dtype tag for the eps block transposes (pure data movement):
TRANSPOSE_DT = F32R  # 1.5 cyc/row vs 2.0 for plain fp32

STAGE_CHUNK = 16  # b-rows per staging buffer / scatter DMA


def _build_program():
    BB = int(os.environ.get("KERNEL_BB", "2"))        # b-rows per eps DMA
    EPS_BUFS = int(os.environ.get("KERNEL_EPS_BUFS", "4"))
    eps_internal = os.environ.get("KERNEL_EPS_INTERNAL", "0") == "1"
    store_gpsimd = os.environ.get("KERNEL_STORE_GPSIMD", "1") == "1"

    nc = bacc.Bacc("TRN2", target_bir_lowering=False, debug=False)

    x_s = nc.dram_tensor("x_s", [B, I], F32, kind="ExternalInput")
    eps_kind = "Internal" if eps_internal else "ExternalInput"
    eps_w_s = nc.dram_tensor("eps_w_s", [B, O, I], TRANSPOSE_DT, kind=eps_kind)
    eps_b_s = nc.dram_tensor("eps_b_s", [B, O], F32, kind="ExternalInput")
    w_mu = nc.dram_tensor("w_mu", [O, I], F32, kind="ExternalInput")
    w_lv = nc.dram_tensor("w_lv", [O, I], F32, kind="ExternalInput")
    b_mu = nc.dram_tensor("b_mu", [1, O], F32, kind="ExternalInput")
    b_lv = nc.dram_tensor("b_lv", [1, O], F32, kind="ExternalInput")
    out_s = nc.dram_tensor("out_s", [B, O], F32, kind="ExternalOutput")

    # Static PSUM: 3 double-bank transpose-staging tensors cycled over
    # half-iterations (n % 3) plus 2 single-bank matvec-row tensors (b % 2).
    # Static allocation avoids Tile pool slot-release waits, which would
    # otherwise push PE matmuls over the 1-sync-wait ISA limit.
    pst_st = [nc.alloc_psum_tensor(f"pst{j}", [P, 2 * O], TRANSPOSE_DT)
              for j in range(3)]
    prow_st = [nc.alloc_psum_tensor(f"prow{j}", [P, O], F32) for j in range(2)]

    with tile.TileContext(nc) as tc:
        with (
            tc.tile_pool(name="consts", bufs=1) as consts,
            tc.tile_pool(name="eps_pool", bufs=EPS_BUFS) as eps_pool,
            tc.tile_pool(name="tp_pool", bufs=2) as tp_pool,
            tc.tile_pool(name="stage_pool", bufs=2) as stage_pool,
        ):
            ident = consts.tile([P, P], F32)
            make_identity(nc, ident[:])
            ident_r = consts.tile([P, P], TRANSPOSE_DT)
            nc.vector.tensor_copy(out=ident_r[:], in_=ident[:])

            # ---- constants / preamble ----------------------------------
            lv_sb = consts.tile([P, OC * I], F32)    # raw logvar [o_loc, c*I+i]
            std_sb = consts.tile([P, OC * I], F32)   # exp(0.5*lv)
            mu_sb = consts.tile([P, OC * I], F32)
            stdT = consts.tile([P, KC * O], F32)     # [i_loc, k*O + o]
            muT = consts.tile([P, KC * O], F32)
            x_sb = consts.tile([P, I], F32)          # [b, i]
            xT = consts.tile([P, KC * P], F32)       # [i_loc, k*P + b]
            xTr = consts.tile([P, KC * P], MATVEC_DT)  # rounded copy for matvec
            epsb_sb = consts.tile([P, O], F32)       # [b, o]
            bmu_row = consts.tile([1, O], F32)
            blv_row = consts.tile([1, O], F32)
            bstd_row = consts.tile([1, O], F32)
            ones_col = consts.tile([1, P], F32)
            U = consts.tile([P, O], F32)             # out1 + bias terms, [b, o]
            G = consts.tile([P, O], F32)             # gathered eps-term rows
            final_sb = consts.tile([P, O], F32)

            nc.sync.dma_start(out=x_sb[:], in_=x_s.ap())
            nc.sync.dma_start(out=epsb_sb[:], in_=eps_b_s.ap())
            nc.sync.dma_start(out=bmu_row[:], in_=b_mu.ap())
            nc.sync.dma_start(out=blv_row[:], in_=b_lv.ap())
            nc.sync.dma_start(
                out=lv_sb[:].rearrange("p (c i) -> p c i", c=OC),
                in_=w_lv.ap().rearrange("(c p) i -> p c i", p=P),
            )
            nc.sync.dma_start(
                out=mu_sb[:].rearrange("p (c i) -> p c i", c=OC),
                in_=w_mu.ap().rearrange("(c p) i -> p c i", p=P),
            )
            nc.vector.memset(ones_col[:], 1.0)

            # std = exp(0.5 * logvar), bias_std likewise (not in place, so
            # consumers depend on one producer engine only)
            nc.scalar.activation(std_sb[:], lv_sb[:],
                                 mybir.ActivationFunctionType.Exp, scale=0.5)
            nc.scalar.activation(bstd_row[:], blv_row[:],
                                 mybir.ActivationFunctionType.Exp, scale=0.5)

            # transpose std and mu: [o, i] -> [i, o] chunks
            ps_stage = pst_st[0].ap().bitcast(F32)
            for src, dst in ((std_sb, stdT), (mu_sb, muT)):
                for k in range(KC):
                    ps = ps_stage[:, (k % 2) * O:(k % 2) * O + O]
                    for c in range(OC):
                        nc.tensor.transpose(
                            out=ps[:, c * P:(c + 1) * P],
                            in_=src[:, c * I + k * P: c * I + (k + 1) * P],
                            identity=ident[:],
                        )
                    nc.vector.tensor_copy(out=dst[:, k * O:(k + 1) * O], in_=ps)

            # transpose x: [b, i] -> [i_loc, k*P + b]
            psx = pst_st[2].ap().bitcast(F32)[:, :KC * P]
            for k in range(KC):
                nc.tensor.transpose(
                    out=psx[:, k * P:(k + 1) * P],
                    in_=x_sb[:, k * P:(k + 1) * P],
                    identity=ident[:],
                )
            nc.vector.tensor_copy(out=xT[:], in_=psx)
            nc.vector.tensor_copy(out=xTr[:], in_=xT[:])

            # out1[b,o] = sum_i x[b,i]*mu[o,i]  (+ bias_mu via K=1 matmul)
            ps_u = pst_st[1].ap().bitcast(F32)[:, :O]
            for k in range(KC):
                nc.tensor.matmul(
                    out=ps_u,
                    lhsT=xT[:, k * P:(k + 1) * P],
                    rhs=muT[:, k * O:(k + 1) * O],
                    start=(k == 0), stop=False,
                )
            nc.tensor.matmul(out=ps_u, lhsT=ones_col[:], rhs=bmu_row[:],
                             start=False, stop=True)

            # broadcast bias_std across partitions, then
            # U = out1 + bias_mu + bias_std * eps_b
            ps_b = pst_st[1].ap().bitcast(F32)[:, O:2 * O]
            nc.tensor.matmul(out=ps_b, lhsT=ones_col[:], rhs=bstd_row[:],
                             start=True, stop=True)
            nc.vector.tensor_tensor(out=U[:], in0=epsb_sb[:], in1=ps_b,
                                    op=mybir.AluOpType.mult)
            nc.vector.tensor_tensor(out=U[:], in0=U[:], in1=ps_u,
                                    op=mybir.AluOpType.add)

            store_eng = nc.gpsimd if store_gpsimd else nc.scalar

            # ---- main loop over batch rows -----------------------------
            def emit_main(_iv=None):
              stage = None
              eps_t = None
              n_loop = int(os.environ.get("KERNEL_NB", B))
              for b in range(n_loop):
                if b % BB == 0:
                    eps_t = eps_pool.tile([P, BB * OC * I], TRANSPOSE_DT,
                                          tag="eps")
                    eng = nc.sync if (b // BB) % 2 == 0 else nc.scalar
                    eng.dma_start(
                        out=eps_t[:].rearrange("p (b c i) -> p b c i",
                                               b=BB, c=OC),
                        in_=eps_w_s.ap()[b:b + BB].rearrange(
                            "b (c p) i -> p b c i", p=P),
                    )
                sub = (b % BB) * OC * I

                t_p = tp_pool.tile([P, KC * O], MATVEC_DT, tag="tp")
                for h in range(2):  # two halves of the i-chunks
                    n = 2 * b + h
                    pst = pst_st[n % 3].ap()
                    for kk in range(2):
                        k = 2 * h + kk
                        for c in range(OC):
                            nc.tensor.transpose(
                                out=pst[:, kk * O + c * P: kk * O + (c + 1) * P],
                                in_=eps_t[:, sub + c * I + k * P:
                                          sub + c * I + (k + 1) * P],
                                identity=ident_r[:],
                            )
                    nc.vector.tensor_tensor(
                        out=t_p[:, h * 2 * O:(h + 1) * 2 * O],
                        in0=pst,
                        in1=stdT[:, h * 2 * O:(h + 1) * 2 * O],
                        op=mybir.AluOpType.mult,
                    )

                prow = prow_st[b % 2].ap()[:1, :]
                for k in range(KC):
                    nc.tensor.matmul(
                        out=prow,
                        lhsT=xTr[:, k * P + b: k * P + b + 1],
                        rhs=t_p[:, k * O:(k + 1) * O],
                        start=(k == 0), stop=(k == KC - 1),
                    )

                ci = b % STAGE_CHUNK
                if ci == 0:
                    stage = stage_pool.tile([1, STAGE_CHUNK * O], F32,
                                            tag="stage")
                nc.scalar.copy(
                    out=stage[0:1, ci * O:(ci + 1) * O], in_=prow)
                if ci == STAGE_CHUNK - 1:
                    g0 = b - (STAGE_CHUNK - 1)
                    store_eng.dma_start(
                        out=G[g0:g0 + STAGE_CHUNK, :],
                        in_=stage[0:1, :],
                    )
                if b % 32 == 31:
                    f0 = b - 31
                    nc.vector.tensor_tensor(
                        out=final_sb[f0:f0 + 32, :],
                        in0=G[f0:f0 + 32, :],
                        in1=U[f0:f0 + 32, :],
                        op=mybir.AluOpType.add,
                    )
                    store_eng.dma_start(
                        out=out_s.ap()[f0:f0 + 32, :],
                        in_=final_sb[f0:f0 + 32, :],
                    )

            repeat = int(os.environ.get("KERNEL_REPEAT", "0"))
            if repeat > 1:
                with tc.For_i(0, repeat, 1):
                    emit_main()
            else:
                emit_main()

    nc.compile()
    return nc


_NC = None


def _get_program():
    global _NC
    if _NC is None:
        _NC = _build_program()
    return _NC


def kernel(**inputs) -> np.ndarray:
    x = np.ascontiguousarray(np.asarray(inputs["x"], dtype=np.float32))
    eps_w = np.ascontiguousarray(np.asarray(inputs["eps_w"], dtype=np.float32))
    eps_b = np.ascontiguousarray(np.asarray(inputs["eps_b"], dtype=np.float32))
    w_mu = np.ascontiguousarray(np.asarray(inputs["weight_mu"], dtype=np.float32))
    w_lv = np.ascontiguousarray(np.asarray(inputs["weight_logvar"], dtype=np.float32))
    b_mu = np.ascontiguousarray(
        np.asarray(inputs["bias_mu"], dtype=np.float32).reshape(1, O))
    b_lv = np.ascontiguousarray(
        np.asarray(inputs["bias_logvar"], dtype=np.float32).reshape(1, O))

    nc = _get_program()
    in_maps = []
    for ci in range(N_CORES):
        sl = slice(ci * B, (ci + 1) * B)
        in_maps.append({
            "x_s": x[sl],
            "eps_w_s": eps_w[sl],
            "eps_b_s": eps_b[sl],
            "w_mu": w_mu,
            "w_lv": w_lv,
            "b_mu": b_mu,
            "b_lv": b_lv,
        })

    res = run_bass_kernel_spmd(nc, in_maps, core_ids=list(range(N_CORES)))
    out = np.concatenate([res.results[ci]["out_s"] for ci in range(N_CORES)],
                         axis=0)
    return out.astype(np.float32)


# revision 5
# speedup vs baseline: 85.9201x; 1.0054x over previous
"""BayesianLinear forward kernel for 8x Trainium2 NeuronCores.

out[b,o] = sum_i (mu[o,i] + std[o,i]*eps_w[b,o,i]) * x[b,i]
           + bias_mu[o] + bias_std[o]*eps_b[b,o]

Shapes (full): x (1024,512) f32, eps_w (1024,512,512) f32, eps_b (1024,512) f32,
weight_mu/logvar (512,512) f32, bias_mu/logvar (512,) f32 -> out (1024,512) f32.

Strategy: data-parallel over batch (128 rows/core). The 1 GiB eps_w stream is
the cost driver (memory-bound). Per core and per batch row b:
  1. DMA eps_w[b] (512x512) into SBUF in natural layout (o on partitions),
     batched BB rows per transfer, alternating the two HWDGE rings.
  2. TensorE transposes the 16 128x128 blocks into PSUM (i on partitions).
  3. VectorE multiplies the transposed blocks by stdT (= exp(.5*logvar)
     transposed, precomputed once) straight out of PSUM into SBUF t'.
  4. TensorE matvec: out2[b,:] accumulates lhsT = x-column (128x1 stationary),
     rhs = t' chunks -> PSUM row (1,512), fp32r for single-pass speed.
  5. ScalarE copies the PSUM row into a partition-0 staging buffer; every 16
     rows one SBUF->SBUF DMA (SWDGE/gpsimd ring) scatters the staging buffer
     to 16 partitions of the gather tile G.
Once per core: out1 = x @ mu^T + bias_mu via 5 batched matmuls into a (b,o)
PSUM tile; bias_std broadcast via a K=1 ones matmul; U = out1 + bias_mu
+ bias_std*eps_b on VectorE.  Final: out = G + U, one DMA to HBM.
"""

import os
import sys

import numpy as np

for _p in ("/opt/trn_rl_repo", "/root/.axon_site/_ro/trn_rl_repo"):
    if os.path.isdir(_p) and _p not in sys.path:
        sys.path.insert(0, _p)

from concourse import bacc, bass, mybir  # noqa: E402
from concourse import tile  # noqa: E402
from concourse.bass_utils import run_bass_kernel_spmd  # noqa: E402
from concourse.masks import make_identity  # noqa: E402

P = 128          # partitions
I = 512          # in_features
O = 512          # out_features
B_FULL = 1024    # full batch
N_CORES = 8
B = B_FULL // N_CORES   # batch rows per core
KC = I // P      # i-chunks
OC = O // P      # o-chunks
F32 = mybir.dt.float32
F32R = mybir.dt.float32r

# dtype used for the per-b matvec (accuracy/speed knob):
#   F32R = single-pass (1 cyc/row), F32 = exact 2-pass (4 cyc/row)
MATVEC_DT = F32R
# dtype tag for the eps block transposes (pure data movement):
TRANSPOSE_DT = F32R  # 1.5 cyc/row vs 2.0 for plain fp32

STAGE_CHUNK = 16  # b-rows per staging buffer / scatter DMA


def _build_program():
    BB = int(os.environ.get("KERNEL_BB", "2"))        # b-rows per eps DMA
    EPS_BUFS = int(os.environ.get("KERNEL_EPS_BUFS", "4"))
    eps_internal = os.environ.get("KERNEL_EPS_INTERNAL", "0") == "1"
    store_gpsimd = os.environ.get("KERNEL_STORE_GPSIMD", "1") == "1"

    nc = bacc.Bacc("TRN2", target_bir_lowering=False, debug=False)

    x_s = nc.dram_tensor("x_s", [B, I], F32, kind="ExternalInput")
    eps_kind = "Internal" if eps_internal else "ExternalInput"
    eps_w_s = nc.dram_tensor("eps_w_s", [B, O, I], TRANSPOSE_DT, kind=eps_kind)
    eps_b_s = nc.dram_tensor("eps_b_s", [B, O], F32, kind="ExternalInput")
    w_mu = nc.dram_tensor("w_mu", [O, I], F32, kind="ExternalInput")
    w_lv = nc.dram_tensor("w_lv", [O, I], F32, kind="ExternalInput")
    b_mu = nc.dram_tensor("b_mu", [1, O], F32, kind="ExternalInput")
    b_lv = nc.dram_tensor("b_lv", [1, O], F32, kind="ExternalInput")
    out_s = nc.dram_tensor("out_s", [B, O], F32, kind="ExternalOutput")

    # Static PSUM: 3 double-bank transpose-staging tensors cycled over
    # half-iterations (n % 3) plus 2 single-bank matvec-row tensors (b % 2).
    # Static allocation avoids Tile pool slot-release waits, which would
    # otherwise push PE matmuls over the 1-sync-wait ISA limit.
    pst_st = [nc.alloc_psum_tensor(f"pst{j}", [P, 2 * O], TRANSPOSE_DT)
              for j in range(3)]
    prow_st = [nc.alloc_psum_tensor(f"prow{j}", [P, O], F32) for j in range(2)]

    with tile.TileContext(nc) as tc:
        with (
            tc.tile_pool(name="consts", bufs=1) as consts,
            tc.tile_pool(name="eps_pool", bufs=EPS_BUFS) as eps_pool,
            tc.tile_pool(name="tp_pool", bufs=2) as tp_pool,
            tc.tile_pool(name="stage_pool", bufs=2) as stage_pool,
        ):
            ident = consts.tile([P, P], F32)
            make_identity(nc, ident[:])
            ident_r = consts.tile([P, P], TRANSPOSE_DT)
            nc.vector.tensor_copy(out=ident_r[:], in_=ident[:])

            # ---- constants / preamble ----------------------------------
            lv_sb = consts.tile([P, OC * I], F32)    # raw logvar [o_loc, c*I+i]
            std_sb = consts.tile([P, OC * I], F32)   # exp(0.5*lv)
            mu_sb = consts.tile([P, OC * I], F32)
            stdT = consts.tile([P, KC * O], F32)     # [i_loc, k*O + o]
            muT = consts.tile([P, KC * O], F32)
            x_sb = consts.tile([P, I], F32)          # [b, i]
            xT = consts.tile([P, KC * P], F32)       # [i_loc, k*P + b]
            xTr = consts.tile([P, KC * P], MATVEC_DT)  # rounded copy for matvec
            epsb_sb = consts.tile([P, O], F32)       # [b, o]
            bmu_row = consts.tile([1, O], F32)
            blv_row = consts.tile([1, O], F32)
            bstd_row = consts.tile([1, O], F32)
            ones_col = consts.tile([1, P], F32)
            U = consts.tile([P, O], F32)             # out1 + bias terms, [b, o]
            G = consts.tile([P, O], F32)             # gathered eps-term rows
            final_sb = consts.tile([P, O], F32)

            nc.sync.dma_start(out=x_sb[:], in_=x_s.ap())
            nc.sync.dma_start(out=epsb_sb[:], in_=eps_b_s.ap())
            nc.sync.dma_start(out=bmu_row[:], in_=b_mu.ap())
            nc.sync.dma_start(out=blv_row[:], in_=b_lv.ap())
            nc.sync.dma_start(
                out=lv_sb[:].rearrange("p (c i) -> p c i", c=OC),
                in_=w_lv.ap().rearrange("(c p) i -> p c i", p=P),
            )
            nc.sync.dma_start(
                out=mu_sb[:].rearrange("p (c i) -> p c i", c=OC),
                in_=w_mu.ap().rearrange("(c p) i -> p c i", p=P),
            )
            nc.vector.memset(ones_col[:], 1.0)

            # std = exp(0.5 * logvar), bias_std likewise (not in place, so
            # consumers depend on one producer engine only)
            nc.scalar.activation(std_sb[:], lv_sb[:],
                                 mybir.ActivationFunctionType.Exp, scale=0.5)
            nc.scalar.activation(bstd_row[:], blv_row[:],
                                 mybir.ActivationFunctionType.Exp, scale=0.5)

            # transpose std and mu: [o, i] -> [i, o] chunks
            ps_stage = pst_st[0].ap().bitcast(F32)
            for src, dst in ((std_sb, stdT), (mu_sb, muT)):
                for k in range(KC):
                    ps = ps_stage[:, (k % 2) * O:(k % 2) * O + O]
                    for c in range(OC):
                        nc.tensor.transpose(
                            out=ps[:, c * P:(c + 1) * P],
                            in_=src[:, c * I + k * P: c * I + (k + 1) * P],
                            identity=ident[:],
                        )
                    nc.vector.tensor_copy(out=dst[:, k * O:(k + 1) * O], in_=ps)

            # transpose x: [b, i] -> [i_loc, k*P + b]
            psx = pst_st[2].ap().bitcast(F32)[:, :KC * P]
            for k in range(KC):
                nc.tensor.transpose(
                    out=psx[:, k * P:(k + 1) * P],
                    in_=x_sb[:, k * P:(k + 1) * P],
                    identity=ident[:],
                )
            nc.vector.tensor_copy(out=xT[:], in_=psx)
            nc.vector.tensor_copy(out=xTr[:], in_=xT[:])

            # out1[b,o] = sum_i x[b,i]*mu[o,i]  (+ bias_mu via K=1 matmul)
            ps_u = pst_st[1].ap().bitcast(F32)[:, :O]
            for k in range(KC):
                nc.tensor.matmul(
                    out=ps_u,
                    lhsT=xT[:, k * P:(k + 1) * P],
                    rhs=muT[:, k * O:(k + 1) * O],
                    start=(k == 0), stop=False,
                )
            nc.tensor.matmul(out=ps_u, lhsT=ones_col[:], rhs=bmu_row[:],
                             start=False, stop=True)

            # broadcast bias_std across partitions, then
            # U = out1 + bias_mu + bias_std * eps_b
            ps_b = pst_st[1].ap().bitcast(F32)[:, O:2 * O]
            nc.tensor.matmul(out=ps_b, lhsT=ones_col[:], rhs=bstd_row[:],
                             start=True, stop=True)
            nc.vector.tensor_tensor(out=U[:], in0=epsb_sb[:], in1=ps_b,
                                    op=mybir.AluOpType.mult)
            nc.vector.tensor_tensor(out=U[:], in0=U[:], in1=ps_u,
                                    op=mybir.AluOpType.add)

            store_eng = nc.gpsimd if store_gpsimd else nc.scalar
            pipe_mv = os.environ.get("KERNEL_PIPE_MV", "1") == "1"

            # ---- main loop over batch rows -----------------------------
            def emit_main(_iv=None):
              state = {"stage": None}
              eps_t = None
              n_loop = int(os.environ.get("KERNEL_NB", B))

              def tail_for(bm, t_prev):
                # matvec + gather/store chain for batch row bm, reading the
                # already-multiplied t_prev.  Emitted one iteration late when
                # pipe_mv so PE's in-order queue never stalls on VectorE.
                prow = prow_st[bm % 2].ap()[:1, :]
                for k in range(KC):
                    nc.tensor.matmul(
                        out=prow,
                        lhsT=xTr[:, k * P + bm: k * P + bm + 1],
                        rhs=t_prev[:, k * O:(k + 1) * O],
                        start=(k == 0), stop=(k == KC - 1),
                    )
                ci = bm % STAGE_CHUNK
                if ci == 0:
                    state["stage"] = stage_pool.tile(
                        [1, STAGE_CHUNK * O], F32, tag="stage", name="stage")
                nc.scalar.copy(
                    out=state["stage"][0:1, ci * O:(ci + 1) * O], in_=prow)
                if ci == STAGE_CHUNK - 1:
                    g0 = bm - (STAGE_CHUNK - 1)
                    store_eng.dma_start(
                        out=G[g0:g0 + STAGE_CHUNK, :],
                        in_=state["stage"][0:1, :],
                    )
                if bm % 32 == 31:
                    f0 = bm - 31
                    nc.vector.tensor_tensor(
                        out=final_sb[f0:f0 + 32, :],
                        in0=G[f0:f0 + 32, :],
                        in1=U[f0:f0 + 32, :],
                        op=mybir.AluOpType.add,
                    )
                    store_eng.dma_start(
                        out=out_s.ap()[f0:f0 + 32, :],
                        in_=final_sb[f0:f0 + 32, :],
                    )

              dma_only = os.environ.get("KERNEL_DMA_ONLY", "0") == "1"
              t_prev = None
              for b in range(n_loop):
                if b % BB == 0:
                    eps_t = eps_pool.tile([P, BB * OC * I], TRANSPOSE_DT,
                                          tag="eps")
                    eng = nc.sync if (b // BB) % 2 == 0 else nc.scalar
                    eng.dma_start(
                        out=eps_t[:].rearrange("p (b c i) -> p b c i",
                                               b=BB, c=OC),
                        in_=eps_w_s.ap()[b:b + BB].rearrange(
                            "b (c p) i -> p b c i", p=P),
                    )
                if dma_only:
                    continue
                sub = (b % BB) * OC * I

                t_p = tp_pool.tile([P, KC * O], MATVEC_DT, tag="tp")
                for h in range(2):  # two halves of the i-chunks
                    n = 2 * b + h
                    pst = pst_st[n % 3].ap()
                    for kk in range(2):
                        k = 2 * h + kk
                        for c in range(OC):
                            nc.tensor.transpose(
                                out=pst[:, kk * O + c * P: kk * O + (c + 1) * P],
                                in_=eps_t[:, sub + c * I + k * P:
                                          sub + c * I + (k + 1) * P],
                                identity=ident_r[:],
                            )
                    nc.vector.tensor_tensor(
                        out=t_p[:, h * 2 * O:(h + 1) * 2 * O],
                        in0=pst,
                        in1=stdT[:, h * 2 * O:(h + 1) * 2 * O],
                        op=mybir.AluOpType.mult,
                    )

                if pipe_mv:
                    if t_prev is not None:
                        tail_for(b - 1, t_prev)
                    t_prev = t_p
                else:
                    tail_for(b, t_p)
              if pipe_mv and t_prev is not None:
                tail_for(n_loop - 1, t_prev)

            repeat = int(os.environ.get("KERNEL_REPEAT", "0"))
            if repeat > 1:
                with tc.For_i(0, repeat, 1):
                    emit_main()
            else:
                emit_main()

    nc.compile()
    return nc


_NC = None


def _get_program():
    global _NC
    if _NC is None:
        _NC = _build_program()
    return _NC


def kernel(**inputs) -> np.ndarray:
    x = np.ascontiguousarray(np.asarray(inputs["x"], dtype=np.float32))
    eps_w = np.ascontiguousarray(np.asarray(inputs["eps_w"], dtype=np.float32))
    eps_b = np.ascontiguousarray(np.asarray(inputs["eps_b"], dtype=np.float32))
    w_mu = np.ascontiguousarray(np.asarray(inputs["weight_mu"], dtype=np.float32))
    w_lv = np.ascontiguousarray(np.asarray(inputs["weight_logvar"], dtype=np.float32))
    b_mu = np.ascontiguousarray(
        np.asarray(inputs["bias_mu"], dtype=np.float32).reshape(1, O))
    b_lv = np.ascontiguousarray(
        np.asarray(inputs["bias_logvar"], dtype=np.float32).reshape(1, O))

    nc = _get_program()
    in_maps = []
    for ci in range(N_CORES):
        sl = slice(ci * B, (ci + 1) * B)
        in_maps.append({
            "x_s": x[sl],
            "eps_w_s": eps_w[sl],
            "eps_b_s": eps_b[sl],
            "w_mu": w_mu,
            "w_lv": w_lv,
            "b_mu": b_mu,
            "b_lv": b_lv,
        })

    res = run_bass_kernel_spmd(nc, in_maps, core_ids=list(range(N_CORES)))
    out = np.concatenate([res.results[ci]["out_s"] for ci in range(N_CORES)],
                         axis=0)
    return out.astype(np.float32)
